# revision 17
# baseline (speedup 1.0000x reference)
"""Trainium2 Bass/Tile kernel for the AttentionModule problem (fp8 version).

Computation (per the reference):
    h_proj  = hidden @ Wa[:, :D].T + ba                       [B, 2E]
    e_proj  = einsum('tbe,fe->tbf', enc, Wa[:, D:])           [T, B, 2E]
    act     = tanh(h_proj + e_proj)
    scores  = einsum('tbf,f->bt', act, w2[0]) (+ b2, dropped — softmax invariant)
    weights = softmax(scores, axis=t)
    applied = einsum('bt,tbe->be', weights, enc)              [B, E]
    out     = tanh(cat(decoder_out, applied) @ Wc.T + bc)     [B, D]

Strategy: data-parallel over batch B=64 across 8 NeuronCores (8 rows each).
The PE-dominant matmuls (e_proj 17.2 GFLOP/core, h_proj, scores) run in
fp8e4 with MatmulPerfMode.DoubleRow (2 contraction rows per PE cell per
cycle -> ~2x bf16 throughput).  Weights (WaE.T, WaD.T, w2) are host-scaled
by 512 so their ~U(+-0.022) values sit in fp8e4 normal range; the 1/512
un-scale is folded into the scalar-engine activation `scale` operand.
Precision-critical paths stay bf16: the applied weighted-sum (DVE
scalar_tensor_tensor on a bf16 copy of enc) and the combine matmul.

Per-core loop structure (BL=8 batch rows, blocks of 2, p-outer weight reuse):
    for block in 4:  for j in 16 f-tiles:  per pair: 1 LDWEIGHTS + 2 batch-row
    stream the same stationary WaET tile (LDW fully hidden).  tanh lands in
    fp8 act pair-tiles; score matmuls (M=1 DoubleRow) interleave lagged.
    Softmax on one partition, weights broadcast via DRAM round-trip, DVE
    reduces applied.  Combine matmul: dec-half mid-loop, applied-half +
    re-injected dec-half (identity-matmul from SBUF) in a pipelined tail
    (per k-tile: DVE applied-reduce -> ACT copy -> PE transpose -> combine
    matmul) so the PE never cools off.
"""

import numpy as np
import ml_dtypes
from contextlib import ExitStack

import concourse.bass as bass
import concourse.tile as tile
from concourse import bacc, mybir
from concourse.bass_utils import run_bass_kernel_spmd
from concourse.masks import make_identity

B, T, E, D = 64, 512, 1024, 1024
NCORES = 8
BL = B // NCORES          # 8 batch rows per core
F = 2 * E                 # 2048
KE = E // 128             # 8 contraction tiles for e/d
NP = KE // 2              # 4 DoubleRow contraction pairs
KC = (D + E) // 128       # 16 contraction tiles for the combine matmul
FJ = F // 128             # 16 f-tiles
NQ = FJ // 2              # 8 f-tile pairs (score contraction)
BLK = 2                   # batch rows per block
S = 512.0                 # fp8 weight pre-scale
RS = 1.0 / S
BF16 = mybir.dt.bfloat16
F32 = mybir.dt.float32
FP8 = mybir.dt.float8e4
AF = mybir.ActivationFunctionType
ALU = mybir.AluOpType
DR = mybir.MatmulPerfMode.DoubleRow

_nc_cache = None


def _load_consts(tc, ctx, ins, uid=""):
    """Load weights + encoder states into SBUF. Returns tile dict."""
    nc = tc.nc
    const = ctx.enter_context(tc.tile_pool(name=f"const{uid}", bufs=1))
    tl = {}
    tl["ones"] = const.tile([1, BL], BF16, name="ones", tag="ones")
    nc.vector.memset(tl["ones"], 1.0)
    tl["ident"] = const.tile([128, 128], F32, name="ident", tag="ident")
    make_identity(nc, tl["ident"])
    tl["identb"] = const.tile([BL, BL], BF16, name="identb", tag="identb")
    nc.vector.tensor_copy(tl["identb"], tl["ident"][0:BL, 0:BL])

    # Weight matrices are loaded in 512-wide f-chunks, DMA-ordered so the
    # PE can start within ~3us of kernel start (h_proj chunk 0 + e_proj j=0
    # need only ~1.5 MiB on-chip).
    tl["hT8"] = const.tile([128, NP, 2, 16], FP8, name="hT8", tag="hT8")
    nc.sync.dma_start(out=tl["hT8"], in_=ins["hT8"])
    tl["baT"] = const.tile([128, FJ], F32, name="baT", tag="baT")
    nc.sync.dma_start(out=tl["baT"], in_=ins["baT"])
    tl["w28"] = const.tile([128, 2, 16], FP8, name="w28", tag="w28")
    nc.sync.dma_start(out=tl["w28"], in_=ins["w28"])

    tl["enc8"] = []
    tl["encb"] = []
    for b in range(BL):
        t8 = const.tile([128, NP, 2, T], FP8, name=f"enc8_{b}", tag=f"enc8_{b}")
        tl["enc8"].append(t8)
        tb = const.tile([128, KE, T], BF16, name=f"encb_{b}", tag=f"encb_{b}")
        tl["encb"].append(tb)
    tl["waD"] = []
    tl["waE"] = []
    for c in range(4):
        tl["waD"].append(const.tile([128, NP, 2, 512], FP8,
                                    name=f"waD{c}", tag=f"waD{c}"))
        tl["waE"].append(const.tile([128, NP, 2, 512], FP8,
                                    name=f"waE{c}", tag=f"waE{c}"))

    def wchunk(name, c):
        return ins[name][c]

    nc.sync.dma_start(out=tl["waE"][0], in_=wchunk("WaE8", 0))
    nc.sync.dma_start(out=tl["enc8"][0], in_=ins["enc8"][0])
    nc.sync.dma_start(out=tl["enc8"][1], in_=ins["enc8"][1])
    nc.sync.dma_start(out=tl["waD"][0], in_=wchunk("WaD8", 0))
    nc.sync.dma_start(out=tl["waD"][1], in_=wchunk("WaD8", 1))
    nc.sync.dma_start(out=tl["waD"][2], in_=wchunk("WaD8", 2))
    nc.sync.dma_start(out=tl["waD"][3], in_=wchunk("WaD8", 3))
    nc.sync.dma_start(out=tl["waE"][1], in_=wchunk("WaE8", 1))
    nc.sync.dma_start(out=tl["enc8"][2], in_=ins["enc8"][2])
    nc.sync.dma_start(out=tl["enc8"][3], in_=ins["enc8"][3])
    for c in range(2, 4):
        nc.sync.dma_start(out=tl["waE"][c], in_=wchunk("WaE8", c))
    for b in range(2):
        nc.sync.dma_start(out=tl["encb"][b], in_=ins["encb"][b])
    for b in range(4, BL):
        nc.sync.dma_start(out=tl["enc8"][b], in_=ins["enc8"][b])
    for b in range(2, BL):
        nc.sync.dma_start(out=tl["encb"][b], in_=ins["encb"][b])

    tl["decT"] = const.tile([128, KE, BL], BF16, name="decT", tag="decT")
    nc.sync.dma_start(out=tl["decT"], in_=ins["decT"])
    tl["wcT"] = []
    for k in range(KC):
        t_w = const.tile([128, D], BF16, name=f"wcT{k}", tag=f"wcT{k}")
        nc.sync.dma_start(out=t_w, in_=ins["WcT"][k * 128:(k + 1) * 128, :])
        tl["wcT"].append(t_w)
    tl["bc"] = const.tile([1, D], BF16, name="bc_sb", tag="bc")
    nc.sync.dma_start(out=tl["bc"], in_=ins["bcR"])
    return tl


def _compute(tc, ctx, tl, wscr, out_d, app_d, uid=""):
    nc = tc.nc
    work = ctx.enter_context(tc.tile_pool(name=f"work{uid}", bufs=1))
    act_pool = ctx.enter_context(tc.tile_pool(name=f"actp{uid}", bufs=3))
    wrep_pool = ctx.enter_context(tc.tile_pool(name=f"wrepp{uid}", bufs=3))
    scr_pool = ctx.enter_context(tc.tile_pool(name=f"scrp{uid}", bufs=2))
    sm_pool = ctx.enter_context(tc.tile_pool(name=f"smp{uid}", bufs=2))
    pe_psum = ctx.enter_context(tc.tile_pool(name=f"pep{uid}", bufs=4, space="PSUM"))
    ps_psum = ctx.enter_context(tc.tile_pool(name=f"psp{uid}", bufs=1, space="PSUM"))
    misc_psum = ctx.enter_context(
        tc.tile_pool(name=f"mip{uid}", bufs=1, space="PSUM"))

    ones, ident = tl["ones"], tl["ident"]

    # appliedT[e_tile][128, BL] accumulators (fp32)
    appT = []
    for k in range(KE):
        appT.append(work.tile([128, BL], F32, name=f"appT{k}", tag=f"appT{k}"))
    decpart = work.tile([BL, D], BF16, name="decpart", tag="decpart")
    h_proj = work.tile([BL, F], F32, name="h_proj", tag="h_proj")
    h_projT = work.tile([128, FJ, BL], F32, name="h_projT", tag="h_projT")

    # act tiles for the first block's j=0/1 (emitted inside the prologue to
    # overlap the weight-chunk DMAs); dict is handed to the main loop
    acts0 = {}

    def eproj_mm(b, bi, j, bl):
        q, w = j // 2, j % 2
        if w == 0:
            acts0[(bi, q)] = act_pool.tile(
                [128, 2, T], FP8, name=f"act{bl}_{bi}_{q}", tag=f"act{bi}")
        pe = pe_psum.tile([128, T], F32, name=f"pe{bl}_{j}_{bi}", tag="pe")
        for p in range(NP):
            nc.tensor.matmul(
                pe,
                tl["waE"][j // 4][:, p, :, (j % 4) * 128:(j % 4 + 1) * 128],
                tl["enc8"][b][:, p, :, :],
                start=(p == 0), stop=(p == NP - 1), perf_mode=DR,
            )
        nc.scalar.activation(acts0[(bi, q)][:, w, :], pe, AF.Tanh,
                             bias=h_projT[:, j, b:b + 1], scale=RS)

    # ---- prologue: h_proj (fp8 DoubleRow, 512x scale) interleaved with the
    # first two e_proj f-tiles of block 0, paced by the weight-chunk DMAs.
    # Emission order is deadlock-aware: ACT is strict FIFO, so every tanh(j)
    # must be emitted after the Identity producing h_projT[:, j] (j=0..3 all
    # come from weight chunk 0).
    def h_chunk(c):
        ph = misc_psum.tile([BL, 512], F32, name=f"ph{c}", tag="misc")
        for p in range(NP):
            nc.tensor.matmul(
                ph, tl["hT8"][:, p, :, 0:BL], tl["waD"][c][:, p, :, :],
                start=(p == 0), stop=(p == NP - 1), perf_mode=DR,
            )
        nc.scalar.copy(h_proj[:, c * 512:(c + 1) * 512], ph)

    def h_transpose(c):
        for j in range(c * 4, (c + 1) * 4):
            pt = misc_psum.tile([128, BL], F32, name=f"pt{j}", tag="misc")
            nc.tensor.transpose(pt, h_proj[:, j * 128:(j + 1) * 128],
                                ident[0:BL, 0:BL])
            nc.scalar.activation(h_projT[:, j, :], pt, AF.Identity,
                                 bias=tl["baT"][:, j:j + 1], scale=RS)

    h_chunk(0)
    h_transpose(0)
    eproj_mm(0, 0, 0, 0)
    eproj_mm(1, 1, 0, 0)
    h_chunk(1)
    eproj_mm(0, 0, 1, 0)
    eproj_mm(1, 1, 1, 0)
    h_transpose(1)
    eproj_mm(0, 0, 2, 0)
    eproj_mm(1, 1, 2, 0)
    h_chunk(2)
    eproj_mm(0, 0, 3, 0)
    eproj_mm(1, 1, 3, 0)
    h_transpose(2)
    h_chunk(3)
    h_transpose(3)

    # ---- main loop: two blocks of BLK batch rows ----
    for bl in range(BL // BLK):
        bs = [bl * BLK + i for i in range(BLK)]
        # (bi, q) -> fp8 act pair tile [128, 2, 512]; block 0 inherits the
        # j=0/1 tiles emitted in the prologue
        acts = acts0 if bl == 0 else {}
        pss = []
        for bi in range(BLK):
            pss.append(ps_psum.tile([1, T], F32, name=f"ps{bl}_{bi}",
                                    tag=f"ps{bi}"))

        j0 = 4 if bl == 0 else 0
        for j in range(j0, FJ):
            q, w = j // 2, j % 2
            pes = []
            for bi, b in enumerate(bs):
                if w == 0:
                    acts[(bi, q)] = act_pool.tile(
                        [128, 2, T], FP8, name=f"act{bl}_{bi}_{q}",
                        tag=f"act{bi}")
                pes.append(pe_psum.tile([128, T], F32,
                                        name=f"pe{bl}_{j}_{bi}", tag="pe"))
            for p in range(NP):
                for bi in range(BLK):
                    nc.tensor.matmul(
                        pes[bi],
                        tl["waE"][j // 4][:, p, :,
                                          (j % 4) * 128:(j % 4 + 1) * 128],
                        tl["enc8"][bs[bi]][:, p, :, :],
                        start=(p == 0), stop=(p == NP - 1), perf_mode=DR,
                    )
            for bi, b in enumerate(bs):
                nc.scalar.activation(acts[(bi, q)][:, w, :], pes[bi], AF.Tanh,
                                     bias=h_projT[:, j, b:b + 1], scale=RS)
            # lag the score matmul one pair behind the act producer
            if w == 1 and q >= 1:
                qs = list(range(q)) if (j == j0 + 1 and j0 > 0) else [q - 1]
                for qq in qs:
                    for bi in range(BLK):
                        nc.tensor.matmul(
                            pss[bi], tl["w28"][:, :, qq:qq + 1],
                            acts[(bi, qq)],
                            start=(qq == 0), stop=False, perf_mode=DR,
                        )
        wreps = {}
        for bi, b in enumerate(bs):
            nc.tensor.matmul(pss[bi], tl["w28"][:, :, NQ - 1:NQ],
                             acts[(bi, NQ - 1)], start=False, stop=True,
                             perf_mode=DR)

            # softmax over t on one partition (scores are 512x true scale;
            # no max-subtraction needed: |scores| << 88*512)
            wexp = sm_pool.tile([1, T], F32, name=f"wexp{b}", tag="wexp")
            sume = sm_pool.tile([1, 1], F32, name=f"sume{b}", tag="sume")
            nc.scalar.activation(wexp, pss[bi], AF.Exp, scale=RS,
                                 accum_out=sume)
            rsum = sm_pool.tile([1, 1], F32, name=f"rsum{b}", tag="rsum")
            nc.vector.reciprocal(rsum, sume)
            wnorm = sm_pool.tile([1, T], BF16, name=f"wnorm{b}", tag="wnorm")
            nc.vector.tensor_scalar_mul(wnorm, wexp, rsum)

            # broadcast weights to 128 partitions via DRAM round-trip
            nc.sync.dma_start(out=wscr[b:b + 1, :], in_=wnorm)
            wrep = wrep_pool.tile([128, T], BF16, name=f"wrep{b}", tag="wrep")
            row = wscr[b:b + 1, :]
            bsrc = bass.AP(tensor=row.tensor, offset=row.offset,
                           ap=[[0, 128]] + [list(p) for p in row.ap[1:]])
            nc.sync.dma_start(out=wrep, in_=bsrc)
            wreps[bi] = wrep

            if bl < BL // BLK - 1:
                # appliedT[:, b] = sum_t enc * w  (DVE, fp32 accum)
                for k in range(KE):
                    scr = scr_pool.tile([128, T], BF16, name=f"scr{b}_{k}",
                                        tag="scr")
                    nc.vector.scalar_tensor_tensor(
                        out=scr, in0=tl["encb"][b][:, k, :], scalar=1.0,
                        in1=wrep, op0=ALU.mult, op1=ALU.mult,
                        accum_out=appT[k][:, b:b + 1],
                    )

        if bl == BL // BLK - 1:
            # ---- pipelined tail: per k-tile, finish the applied reduction
            # for the last block, then immediately transpose + feed the
            # combine matmul so PE never cools off ----
            applied_sb = work.tile([BL, E], F32, name="applied_sb",
                                   tag="applied_sb")
            appT_bf = []
            pc0 = ps_psum.tile([BL, 512], F32, name="pc0", tag="ps0")
            for k in range(KE):
                for bi, b in enumerate(bs):
                    scr = scr_pool.tile([128, T], BF16, name=f"scrt{b}_{k}",
                                        tag="scr")
                    nc.vector.scalar_tensor_tensor(
                        out=scr, in0=tl["encb"][b][:, k, :], scalar=1.0,
                        in1=wreps[bi], op0=ALU.mult, op1=ALU.mult,
                        accum_out=appT[k][:, b:b + 1],
                    )
                t_c = work.tile([128, BL], BF16, name=f"appBf{k}",
                                tag=f"appBf{k}")
                nc.scalar.copy(t_c, appT[k])
                appT_bf.append(t_c)
                pa = misc_psum.tile([BL, 128], F32, name=f"pa{k}", tag="misc")
                nc.tensor.transpose(pa, appT[k], ident)
                nc.scalar.copy(applied_sb[:, k * 128:(k + 1) * 128], pa)
                nc.tensor.matmul(pc0, t_c, tl["wcT"][KE + k][:, 0:512],
                                 start=(k == 0), stop=False)
            nc.sync.dma_start(out=app_d, in_=applied_sb)

            out_sb = work.tile([BL, D], F32, name="out_sb", tag="out_sb")
            nc.tensor.matmul(pc0, tl["identb"], decpart[:, 0:512],
                             start=False, stop=False)
            nc.tensor.matmul(pc0, ones, tl["bc"][:, 0:512], start=False,
                             stop=True)
            nc.scalar.activation(out_sb[:, 0:512], pc0, AF.Tanh)
            pc1 = ps_psum.tile([BL, 512], F32, name="pc1", tag="ps1")
            for k in range(KE):
                nc.tensor.matmul(pc1, appT_bf[k],
                                 tl["wcT"][KE + k][:, 512:1024],
                                 start=(k == 0), stop=False)
            nc.tensor.matmul(pc1, tl["identb"], decpart[:, 512:1024],
                             start=False, stop=False)
            nc.tensor.matmul(pc1, ones, tl["bc"][:, 512:1024], start=False,
                             stop=True)
            nc.scalar.activation(out_sb[:, 512:1024], pc1, AF.Tanh)
            nc.sync.dma_start(out=out_d, in_=out_sb)

        # between blocks: decoder half of the combine matmul (kept off the
        # tail; result parked in SBUF bf16 and re-injected in the epilogue)
        if bl == 1:
            for h in range(D // 512):
                pc = misc_psum.tile([BL, 512], F32, name=f"pcd{h}", tag="misc")
                for k in range(KE):
                    nc.tensor.matmul(
                        pc, tl["decT"][:, k, :],
                        tl["wcT"][k][:, h * 512:(h + 1) * 512],
                        start=(k == 0), stop=(k == KE - 1),
                    )
                nc.scalar.copy(decpart[:, h * 512:(h + 1) * 512], pc)



def build_nc(reps=1, mode="fp8"):
    nc = bacc.Bacc("TRN2", target_bir_lowering=False, debug=False)
    ins = {}

    def din(name, shape, dt):
        ins[name] = nc.dram_tensor(name, shape, dt, kind="ExternalInput").ap()

    din("enc8", [BL, 128, NP, 2, T], FP8)
    din("encb", [BL, 128, KE, T], BF16)
    din("hT8", [128, NP, 2, 16], FP8)
    din("baT", [128, FJ], F32)
    din("WaD8", [4, 128, NP, 2, 512], FP8)
    din("WaE8", [4, 128, NP, 2, 512], FP8)
    din("w28", [128, 2, 16], FP8)
    din("decT", [128, KE, BL], BF16)
    din("WcT", [D + E, D], BF16)
    din("bcR", [1, D], BF16)
    wscr = nc.dram_tensor("wscr", [BL, T], BF16, kind="Internal").ap()
    out_d = nc.dram_tensor("out", [BL, D], F32, kind="ExternalOutput").ap()
    app_d = nc.dram_tensor("applied", [BL, E], F32, kind="ExternalOutput").ap()
    with tile.TileContext(nc) as tc:
        for r in range(reps):
            with ExitStack() as ctx:
                tl = _load_consts(tc, ctx, ins, uid=f"r{r}")
                _compute(tc, ctx, tl, wscr, out_d, app_d, uid=f"r{r}")
    nc.compile()
    return nc


def _prep_inputs(hidden, decoder_out, encoder_states, Wa, ba, w2, Wc, bc):
    bf = ml_dtypes.bfloat16
    f8 = ml_dtypes.float8_e4m3
    f32 = np.float32

    def to8(a, s=1.0):
        return np.ascontiguousarray(
            np.clip(np.asarray(a, f32) * s, -240.0, 240.0)).astype(f8)

    def pair4(mT):  # [1024, N] -> [128, 4, 2, N] pair-interleaved
        n = mT.shape[1]
        return np.ascontiguousarray(
            mT.reshape(NP, 2, 128, n).transpose(2, 0, 1, 3))

    def chunk4(a):  # [128, NP, 2, F] -> [4, 128, NP, 2, 512] contiguous
        return np.ascontiguousarray(np.stack(
            [a[:, :, :, c * 512:(c + 1) * 512] for c in range(4)]))

    WaD = np.asarray(Wa[:, :D], f32)
    WaE = np.asarray(Wa[:, D:], f32)
    shared = {
        "WaD8": to8(chunk4(pair4(WaD.T)), S),
        "WaE8": to8(chunk4(pair4(WaE.T)), S),
        "w28": to8(np.concatenate([
            np.asarray(w2[0], f32).reshape(NQ, 2, 128).transpose(2, 1, 0),
            np.zeros((128, 2, 16 - NQ), f32)], axis=2), S),
        "baT": np.ascontiguousarray(
            np.asarray(ba, f32).reshape(FJ, 128).T),
        "WcT": np.ascontiguousarray(np.asarray(Wc, f32).T).astype(bf),
        "bcR": np.asarray(bc, f32).reshape(1, D).astype(bf),
    }
    enc_f = np.asarray(encoder_states, f32)  # [T, B, E]
    in_maps = []
    for c in range(NCORES):
        sl = slice(c * BL, (c + 1) * BL)
        encT = enc_f[:, sl, :].transpose(1, 2, 0)   # [BL, E, T]
        m = dict(shared)
        m["enc8"] = to8(np.ascontiguousarray(
            encT.reshape(BL, NP, 2, 128, T).transpose(0, 3, 1, 2, 4)))
        m["encb"] = np.ascontiguousarray(
            encT.reshape(BL, KE, 128, T).transpose(0, 2, 1, 3)).astype(bf)
        hT = pair4(np.asarray(hidden[sl], f32).T)  # [128, NP, 2, BL]
        m["hT8"] = to8(np.concatenate(
            [hT, np.zeros((128, NP, 2, 16 - BL), f32)], axis=3))
        m["decT"] = np.ascontiguousarray(
            np.asarray(decoder_out[sl], f32).T.reshape(KE, 128, BL)
            .transpose(1, 0, 2)).astype(bf)
        in_maps.append(m)
    return in_maps


def kernel(hidden, decoder_out, encoder_states, Wa, ba, w2, b2, Wc, bc):
    global _nc_cache
    if _nc_cache is None:
        _nc_cache = build_nc()
    in_maps = _prep_inputs(hidden, decoder_out, encoder_states, Wa, ba, w2, Wc, bc)
    res = run_bass_kernel_spmd(_nc_cache, in_maps, core_ids=list(range(NCORES)))
    out = np.concatenate([res.results[c]["out"] for c in range(NCORES)], axis=0)
    applied = np.concatenate(
        [res.results[c]["applied"] for c in range(NCORES)], axis=0)
    return out.astype(np.float32), applied.astype(np.float32)


# revision 18
# speedup vs baseline: 1.7524x; 1.7524x over previous
"""Trainium2 Bass/Tile kernel for the AttentionModule problem (fp8 version).

Computation (per the reference):
    h_proj  = hidden @ Wa[:, :D].T + ba                       [B, 2E]
    e_proj  = einsum('tbe,fe->tbf', enc, Wa[:, D:])           [T, B, 2E]
    act     = tanh(h_proj + e_proj)
    scores  = einsum('tbf,f->bt', act, w2[0]) (+ b2, dropped — softmax invariant)
    weights = softmax(scores, axis=t)
    applied = einsum('bt,tbe->be', weights, enc)              [B, E]
    out     = tanh(cat(decoder_out, applied) @ Wc.T + bc)     [B, D]

Strategy: data-parallel over batch B=64 across 8 NeuronCores (8 rows each).
The PE-dominant matmuls (e_proj 17.2 GFLOP/core, h_proj, scores) run in
fp8e4 with MatmulPerfMode.DoubleRow (2 contraction rows per PE cell per
cycle -> ~2x bf16 throughput).  Weights (WaE.T, WaD.T, w2) are host-scaled
by 512 so their ~U(+-0.022) values sit in fp8e4 normal range; the 1/512
un-scale is folded into the scalar-engine activation `scale` operand.
Precision-critical paths stay bf16: the applied weighted-sum (DVE
scalar_tensor_tensor on a bf16 copy of enc) and the combine matmul.

Per-core loop structure (BL=8 batch rows, blocks of 2, p-outer weight reuse):
    for block in 4:  for j in 16 f-tiles:  per pair: 1 LDWEIGHTS + 2 batch-row
    stream the same stationary WaET tile (LDW fully hidden).  tanh lands in
    fp8 act pair-tiles; score matmuls (M=1 DoubleRow) interleave lagged.
    Softmax on one partition, weights broadcast via DRAM round-trip, DVE
    reduces applied.  Combine matmul: dec-half mid-loop, applied-half +
    re-injected dec-half (identity-matmul from SBUF) in a pipelined tail
    (per k-tile: DVE applied-reduce -> ACT copy -> PE transpose -> combine
    matmul) so the PE never cools off.
"""

import numpy as np
import ml_dtypes
from contextlib import ExitStack

import concourse.bass as bass
import concourse.tile as tile
from concourse import bacc, mybir
from concourse.bass_utils import run_bass_kernel_spmd
from concourse.masks import make_identity

B, T, E, D = 64, 512, 1024, 1024
NCORES = 8
BL = B // NCORES          # 8 batch rows per core
F = 2 * E                 # 2048
KE = E // 128             # 8 contraction tiles for e/d
NP = KE // 2              # 4 DoubleRow contraction pairs
KC = (D + E) // 128       # 16 contraction tiles for the combine matmul
FJ = F // 128             # 16 f-tiles
NQ = FJ // 2              # 8 f-tile pairs (score contraction)
BLK = 2                   # batch rows per block
S = 512.0                 # fp8 weight pre-scale
RS = 1.0 / S
BF16 = mybir.dt.bfloat16
F32 = mybir.dt.float32
FP8 = mybir.dt.float8e4
AF = mybir.ActivationFunctionType
ALU = mybir.AluOpType
DR = mybir.MatmulPerfMode.DoubleRow

_nc_cache = None


def _load_consts(tc, ctx, ins, uid=""):
    """Load weights + encoder states into SBUF. Returns tile dict."""
    nc = tc.nc
    const = ctx.enter_context(tc.tile_pool(name=f"const{uid}", bufs=1))
    tl = {}
    tl["ones"] = const.tile([1, BL], BF16, name="ones", tag="ones")
    nc.vector.memset(tl["ones"], 1.0)
    tl["ident"] = const.tile([128, 128], F32, name="ident", tag="ident")
    make_identity(nc, tl["ident"])
    tl["identb"] = const.tile([BL, BL], BF16, name="identb", tag="identb")
    nc.vector.tensor_copy(tl["identb"], tl["ident"][0:BL, 0:BL])

    # Weight matrices are loaded in 512-wide f-chunks, DMA-ordered so the
    # PE can start within ~3us of kernel start (h_proj chunk 0 + e_proj j=0
    # need only ~1.5 MiB on-chip).
    tl["hT8"] = const.tile([128, NP, 2, 16], FP8, name="hT8", tag="hT8")
    nc.sync.dma_start(out=tl["hT8"], in_=ins["hT8"])
    tl["baT"] = const.tile([128, FJ], F32, name="baT", tag="baT")
    nc.sync.dma_start(out=tl["baT"], in_=ins["baT"])
    tl["w28"] = const.tile([128, 2, 16], FP8, name="w28", tag="w28")
    nc.sync.dma_start(out=tl["w28"], in_=ins["w28"])

    tl["enc8"] = []
    tl["encb"] = []
    for b in range(BL):
        t8 = const.tile([128, NP, 2, T], FP8, name=f"enc8_{b}", tag=f"enc8_{b}")
        tl["enc8"].append(t8)
        tb = const.tile([128, KE, T], BF16, name=f"encb_{b}", tag=f"encb_{b}")
        tl["encb"].append(tb)
    tl["waD"] = []
    tl["waE"] = []
    for c in range(4):
        tl["waD"].append(const.tile([128, NP, 2, 512], FP8,
                                    name=f"waD{c}", tag=f"waD{c}"))
        tl["waE"].append(const.tile([128, NP, 2, 512], FP8,
                                    name=f"waE{c}", tag=f"waE{c}"))

    def wchunk(name, c):
        return ins[name][c]

    nc.sync.dma_start(out=tl["waD"][0], in_=wchunk("WaD8", 0))
    nc.sync.dma_start(out=tl["waE"][0], in_=wchunk("WaE8", 0))
    nc.sync.dma_start(out=tl["enc8"][0], in_=ins["enc8"][0])
    nc.sync.dma_start(out=tl["enc8"][1], in_=ins["enc8"][1])
    nc.sync.dma_start(out=tl["waD"][1], in_=wchunk("WaD8", 1))
    nc.sync.dma_start(out=tl["waD"][2], in_=wchunk("WaD8", 2))
    nc.sync.dma_start(out=tl["waD"][3], in_=wchunk("WaD8", 3))
    nc.sync.dma_start(out=tl["waE"][1], in_=wchunk("WaE8", 1))
    nc.sync.dma_start(out=tl["enc8"][2], in_=ins["enc8"][2])
    nc.sync.dma_start(out=tl["enc8"][3], in_=ins["enc8"][3])
    for c in range(2, 4):
        nc.sync.dma_start(out=tl["waE"][c], in_=wchunk("WaE8", c))
    for b in range(2):
        nc.sync.dma_start(out=tl["encb"][b], in_=ins["encb"][b])
    for b in range(4, BL):
        nc.sync.dma_start(out=tl["enc8"][b], in_=ins["enc8"][b])
    for b in range(2, BL):
        nc.sync.dma_start(out=tl["encb"][b], in_=ins["encb"][b])

    tl["decT"] = const.tile([128, KE, BL], BF16, name="decT", tag="decT")
    nc.sync.dma_start(out=tl["decT"], in_=ins["decT"])
    tl["wcT"] = []
    for k in range(KC):
        t_w = const.tile([128, D], BF16, name=f"wcT{k}", tag=f"wcT{k}")
        nc.sync.dma_start(out=t_w, in_=ins["WcT"][k * 128:(k + 1) * 128, :])
        tl["wcT"].append(t_w)
    tl["bc"] = const.tile([1, D], BF16, name="bc_sb", tag="bc")
    nc.sync.dma_start(out=tl["bc"], in_=ins["bcR"])
    return tl


def _compute(tc, ctx, tl, wscr, out_d, app_d, uid=""):
    nc = tc.nc
    work = ctx.enter_context(tc.tile_pool(name=f"work{uid}", bufs=1))
    act_pool = ctx.enter_context(tc.tile_pool(name=f"actp{uid}", bufs=3))
    wrep_pool = ctx.enter_context(tc.tile_pool(name=f"wrepp{uid}", bufs=3))
    scr_pool = ctx.enter_context(tc.tile_pool(name=f"scrp{uid}", bufs=2))
    sm_pool = ctx.enter_context(tc.tile_pool(name=f"smp{uid}", bufs=2))
    pe_psum = ctx.enter_context(tc.tile_pool(name=f"pep{uid}", bufs=4, space="PSUM"))
    ps_psum = ctx.enter_context(tc.tile_pool(name=f"psp{uid}", bufs=1, space="PSUM"))
    misc_psum = ctx.enter_context(
        tc.tile_pool(name=f"mip{uid}", bufs=1, space="PSUM"))

    ones, ident = tl["ones"], tl["ident"]

    # appliedT[e_tile][128, BL] accumulators (fp32)
    appT = []
    for k in range(KE):
        appT.append(work.tile([128, BL], F32, name=f"appT{k}", tag=f"appT{k}"))
    decpart = work.tile([BL, D], BF16, name="decpart", tag="decpart")
    h_proj = work.tile([BL, F], F32, name="h_proj", tag="h_proj")
    h_projT = work.tile([128, FJ, BL], F32, name="h_projT", tag="h_projT")

    # act tiles for the first block's j=0/1 (emitted inside the prologue to
    # overlap the weight-chunk DMAs); dict is handed to the main loop
    acts0 = {}

    def eproj_mm(b, bi, j, bl):
        q, w = j // 2, j % 2
        if w == 0:
            acts0[(bi, q)] = act_pool.tile(
                [128, 2, T], FP8, name=f"act{bl}_{bi}_{q}", tag=f"act{bi}")
        pe = pe_psum.tile([128, T], F32, name=f"pe{bl}_{j}_{bi}", tag="pe")
        for p in range(NP):
            nc.tensor.matmul(
                pe,
                tl["waE"][j // 4][:, p, :, (j % 4) * 128:(j % 4 + 1) * 128],
                tl["enc8"][b][:, p, :, :],
                start=(p == 0), stop=(p == NP - 1), perf_mode=DR,
            )
        nc.scalar.activation(acts0[(bi, q)][:, w, :], pe, AF.Tanh,
                             bias=h_projT[:, j, b:b + 1], scale=RS)

    # ---- prologue: h_proj (fp8 DoubleRow, 512x scale) interleaved with the
    # first two e_proj f-tiles of block 0, paced by the weight-chunk DMAs.
    # Emission order is deadlock-aware: ACT is strict FIFO, so every tanh(j)
    # must be emitted after the Identity producing h_projT[:, j] (j=0..3 all
    # come from weight chunk 0).
    def h_chunk(c):
        ph = misc_psum.tile([BL, 512], F32, name=f"ph{c}", tag="misc")
        for p in range(NP):
            nc.tensor.matmul(
                ph, tl["hT8"][:, p, :, 0:BL], tl["waD"][c][:, p, :, :],
                start=(p == 0), stop=(p == NP - 1), perf_mode=DR,
            )
        nc.scalar.copy(h_proj[:, c * 512:(c + 1) * 512], ph)

    def h_transpose(c):
        for j in range(c * 4, (c + 1) * 4):
            pt = misc_psum.tile([128, BL], F32, name=f"pt{j}", tag="misc")
            nc.tensor.transpose(pt, h_proj[:, j * 128:(j + 1) * 128],
                                ident[0:BL, 0:BL])
            nc.scalar.activation(h_projT[:, j, :], pt, AF.Identity,
                                 bias=tl["baT"][:, j:j + 1], scale=RS)

    h_chunk(0)
    h_transpose(0)
    eproj_mm(0, 0, 0, 0)
    eproj_mm(1, 1, 0, 0)
    h_chunk(1)
    eproj_mm(0, 0, 1, 0)
    eproj_mm(1, 1, 1, 0)
    h_transpose(1)
    eproj_mm(0, 0, 2, 0)
    eproj_mm(1, 1, 2, 0)
    h_chunk(2)
    eproj_mm(0, 0, 3, 0)
    eproj_mm(1, 1, 3, 0)
    h_transpose(2)
    h_chunk(3)
    h_transpose(3)

    # ---- main loop: two blocks of BLK batch rows ----
    for bl in range(BL // BLK):
        bs = [bl * BLK + i for i in range(BLK)]
        # (bi, q) -> fp8 act pair tile [128, 2, 512]; block 0 inherits the
        # j=0/1 tiles emitted in the prologue
        acts = acts0 if bl == 0 else {}
        pss = []
        for bi in range(BLK):
            pss.append(ps_psum.tile([1, T], F32, name=f"ps{bl}_{bi}",
                                    tag=f"ps{bi}"))

        j0 = 4 if bl == 0 else 0
        for j in range(j0, FJ):
            q, w = j // 2, j % 2
            pes = []
            for bi, b in enumerate(bs):
                if w == 0:
                    acts[(bi, q)] = act_pool.tile(
                        [128, 2, T], FP8, name=f"act{bl}_{bi}_{q}",
                        tag=f"act{bi}")
                pes.append(pe_psum.tile([128, T], F32,
                                        name=f"pe{bl}_{j}_{bi}", tag="pe"))
            for p in range(NP):
                for bi in range(BLK):
                    nc.tensor.matmul(
                        pes[bi],
                        tl["waE"][j // 4][:, p, :,
                                          (j % 4) * 128:(j % 4 + 1) * 128],
                        tl["enc8"][bs[bi]][:, p, :, :],
                        start=(p == 0), stop=(p == NP - 1), perf_mode=DR,
                    )
            for bi, b in enumerate(bs):
                nc.scalar.activation(acts[(bi, q)][:, w, :], pes[bi], AF.Tanh,
                                     bias=h_projT[:, j, b:b + 1], scale=RS)
            # lag the score matmul one pair behind the act producer
            if w == 1 and q >= 1:
                qs = list(range(q)) if (j == j0 + 1 and j0 > 0) else [q - 1]
                for qq in qs:
                    for bi in range(BLK):
                        nc.tensor.matmul(
                            pss[bi], tl["w28"][:, :, qq:qq + 1],
                            acts[(bi, qq)],
                            start=(qq == 0), stop=False, perf_mode=DR,
                        )
        wreps = {}
        for bi, b in enumerate(bs):
            nc.tensor.matmul(pss[bi], tl["w28"][:, :, NQ - 1:NQ],
                             acts[(bi, NQ - 1)], start=False, stop=True,
                             perf_mode=DR)

            # softmax over t on one partition (scores are 512x true scale;
            # no max-subtraction needed: |scores| << 88*512)
            wexp = sm_pool.tile([1, T], F32, name=f"wexp{b}", tag="wexp")
            sume = sm_pool.tile([1, 1], F32, name=f"sume{b}", tag="sume")
            nc.scalar.activation(wexp, pss[bi], AF.Exp, scale=RS,
                                 accum_out=sume)
            rsum = sm_pool.tile([1, 1], F32, name=f"rsum{b}", tag="rsum")
            nc.vector.reciprocal(rsum, sume)
            wnorm = sm_pool.tile([1, T], BF16, name=f"wnorm{b}", tag="wnorm")
            nc.vector.tensor_scalar_mul(wnorm, wexp, rsum)

            # broadcast weights to 128 partitions via DRAM round-trip
            nc.sync.dma_start(out=wscr[b:b + 1, :], in_=wnorm)
            wrep = wrep_pool.tile([128, T], BF16, name=f"wrep{b}", tag="wrep")
            row = wscr[b:b + 1, :]
            bsrc = bass.AP(tensor=row.tensor, offset=row.offset,
                           ap=[[0, 128]] + [list(p) for p in row.ap[1:]])
            nc.sync.dma_start(out=wrep, in_=bsrc)
            wreps[bi] = wrep

            if bl < BL // BLK - 1:
                # appliedT[:, b] = sum_t enc * w  (DVE, fp32 accum)
                for k in range(KE):
                    scr = scr_pool.tile([128, T], BF16, name=f"scr{b}_{k}",
                                        tag="scr")
                    nc.vector.scalar_tensor_tensor(
                        out=scr, in0=tl["encb"][b][:, k, :], scalar=1.0,
                        in1=wrep, op0=ALU.mult, op1=ALU.mult,
                        accum_out=appT[k][:, b:b + 1],
                    )

        if bl == BL // BLK - 1:
            # ---- pipelined tail: per k-tile, finish the applied reduction
            # for the last block, then immediately transpose + feed the
            # combine matmul so PE never cools off ----
            applied_sb = work.tile([BL, E], F32, name="applied_sb",
                                   tag="applied_sb")
            appT_bf = []
            pc0 = ps_psum.tile([BL, 512], F32, name="pc0", tag="ps0")
            for k in range(KE):
                for bi, b in enumerate(bs):
                    scr = scr_pool.tile([128, T], BF16, name=f"scrt{b}_{k}",
                                        tag="scr")
                    nc.vector.scalar_tensor_tensor(
                        out=scr, in0=tl["encb"][b][:, k, :], scalar=1.0,
                        in1=wreps[bi], op0=ALU.mult, op1=ALU.mult,
                        accum_out=appT[k][:, b:b + 1],
                    )
                t_c = work.tile([128, BL], BF16, name=f"appBf{k}",
                                tag=f"appBf{k}")
                nc.scalar.copy(t_c, appT[k])
                appT_bf.append(t_c)
                pa = misc_psum.tile([BL, 128], F32, name=f"pa{k}", tag="misc")
                nc.tensor.transpose(pa, appT[k], ident)
                nc.scalar.copy(applied_sb[:, k * 128:(k + 1) * 128], pa)
                nc.tensor.matmul(pc0, t_c, tl["wcT"][KE + k][:, 0:512],
                                 start=(k == 0), stop=False)
            nc.sync.dma_start(out=app_d, in_=applied_sb)

            out_sb = work.tile([BL, D], F32, name="out_sb", tag="out_sb")
            nc.tensor.matmul(pc0, tl["identb"], decpart[:, 0:512],
                             start=False, stop=False)
            nc.tensor.matmul(pc0, ones, tl["bc"][:, 0:512], start=False,
                             stop=True)
            nc.scalar.activation(out_sb[:, 0:512], pc0, AF.Tanh)
            pc1 = ps_psum.tile([BL, 512], F32, name="pc1", tag="ps1")
            for k in range(KE):
                nc.tensor.matmul(pc1, appT_bf[k],
                                 tl["wcT"][KE + k][:, 512:1024],
                                 start=(k == 0), stop=False)
            nc.tensor.matmul(pc1, tl["identb"], decpart[:, 512:1024],
                             start=False, stop=False)
            nc.tensor.matmul(pc1, ones, tl["bc"][:, 512:1024], start=False,
                             stop=True)
            nc.scalar.activation(out_sb[:, 512:1024], pc1, AF.Tanh)
            nc.sync.dma_start(out=out_d, in_=out_sb)

        # between blocks: decoder half of the combine matmul (kept off the
        # tail; result parked in SBUF bf16 and re-injected in the epilogue)
        if bl == 1:
            for h in range(D // 512):
                pc = misc_psum.tile([BL, 512], F32, name=f"pcd{h}", tag="misc")
                for k in range(KE):
                    nc.tensor.matmul(
                        pc, tl["decT"][:, k, :],
                        tl["wcT"][k][:, h * 512:(h + 1) * 512],
                        start=(k == 0), stop=(k == KE - 1),
                    )
                nc.scalar.copy(decpart[:, h * 512:(h + 1) * 512], pc)



def build_nc(reps=1, mode="fp8"):
    nc = bacc.Bacc("TRN2", target_bir_lowering=False, debug=False)
    ins = {}

    def din(name, shape, dt):
        ins[name] = nc.dram_tensor(name, shape, dt, kind="ExternalInput").ap()

    din("enc8", [BL, 128, NP, 2, T], FP8)
    din("encb", [BL, 128, KE, T], BF16)
    din("hT8", [128, NP, 2, 16], FP8)
    din("baT", [128, FJ], F32)
    din("WaD8", [4, 128, NP, 2, 512], FP8)
    din("WaE8", [4, 128, NP, 2, 512], FP8)
    din("w28", [128, 2, 16], FP8)
    din("decT", [128, KE, BL], BF16)
    din("WcT", [D + E, D], BF16)
    din("bcR", [1, D], BF16)
    wscr = nc.dram_tensor("wscr", [BL, T], BF16, kind="Internal").ap()
    out_d = nc.dram_tensor("out", [BL, D], F32, kind="ExternalOutput").ap()
    app_d = nc.dram_tensor("applied", [BL, E], F32, kind="ExternalOutput").ap()
    with tile.TileContext(nc) as tc:
        for r in range(reps):
            with ExitStack() as ctx:
                tl = _load_consts(tc, ctx, ins, uid=f"r{r}")
                _compute(tc, ctx, tl, wscr, out_d, app_d, uid=f"r{r}")
    nc.compile()
    return nc


def _prep_inputs(hidden, decoder_out, encoder_states, Wa, ba, w2, Wc, bc):
    bf = ml_dtypes.bfloat16
    f8 = ml_dtypes.float8_e4m3
    f32 = np.float32

    def to8(a, s=1.0):
        return np.ascontiguousarray(
            np.clip(np.asarray(a, f32) * s, -240.0, 240.0)).astype(f8)

    def pair4(mT):  # [1024, N] -> [128, 4, 2, N] pair-interleaved
        n = mT.shape[1]
        return np.ascontiguousarray(
            mT.reshape(NP, 2, 128, n).transpose(2, 0, 1, 3))

    def chunk4(a):  # [128, NP, 2, F] -> [4, 128, NP, 2, 512] contiguous
        return np.ascontiguousarray(np.stack(
            [a[:, :, :, c * 512:(c + 1) * 512] for c in range(4)]))

    WaD = np.asarray(Wa[:, :D], f32)
    WaE = np.asarray(Wa[:, D:], f32)
    shared = {
        "WaD8": to8(chunk4(pair4(WaD.T)), S),
        "WaE8": to8(chunk4(pair4(WaE.T)), S),
        "w28": to8(np.concatenate([
            np.asarray(w2[0], f32).reshape(NQ, 2, 128).transpose(2, 1, 0),
            np.zeros((128, 2, 16 - NQ), f32)], axis=2), S),
        "baT": np.ascontiguousarray(
            np.asarray(ba, f32).reshape(FJ, 128).T),
        "WcT": np.ascontiguousarray(np.asarray(Wc, f32).T).astype(bf),
        "bcR": np.asarray(bc, f32).reshape(1, D).astype(bf),
    }
    enc_f = np.asarray(encoder_states, f32)  # [T, B, E]
    in_maps = []
    for c in range(NCORES):
        sl = slice(c * BL, (c + 1) * BL)
        encT = enc_f[:, sl, :].transpose(1, 2, 0)   # [BL, E, T]
        m = dict(shared)
        m["enc8"] = to8(np.ascontiguousarray(
            encT.reshape(BL, NP, 2, 128, T).transpose(0, 3, 1, 2, 4)))
        m["encb"] = np.ascontiguousarray(
            encT.reshape(BL, KE, 128, T).transpose(0, 2, 1, 3)).astype(bf)
        hT = pair4(np.asarray(hidden[sl], f32).T)  # [128, NP, 2, BL]
        m["hT8"] = to8(np.concatenate(
            [hT, np.zeros((128, NP, 2, 16 - BL), f32)], axis=3))
        m["decT"] = np.ascontiguousarray(
            np.asarray(decoder_out[sl], f32).T.reshape(KE, 128, BL)
            .transpose(1, 0, 2)).astype(bf)
        in_maps.append(m)
    return in_maps


def kernel(hidden, decoder_out, encoder_states, Wa, ba, w2, b2, Wc, bc):
    global _nc_cache
    if _nc_cache is None:
        _nc_cache = build_nc()
    in_maps = _prep_inputs(hidden, decoder_out, encoder_states, Wa, ba, w2, Wc, bc)
    res = run_bass_kernel_spmd(_nc_cache, in_maps, core_ids=list(range(NCORES)))
    out = np.concatenate([res.results[c]["out"] for c in range(NCORES)], axis=0)
    applied = np.concatenate(
        [res.results[c]["applied"] for c in range(NCORES)], axis=0)
    return out.astype(np.float32), applied.astype(np.float32)
